# revision 46
# baseline (speedup 1.0000x reference)
"""Trainium2 Bass kernel for nn_DeepGATEncoder (3-layer GAT + mean-pool + MLP).

Sharding: node rows split 384/core across 8 cores; weights replicated.
Per GAT layer each core computes Wh (+ fused a_dst / a_src columns) for its
own 384 nodes for all 10 heads; the per-head [ones|Wh|d] blocks are
AllGather'ed in batched collectives ([1,1,2,3,3] heads -- small first so
the first att head unblocks early), then each core runs masked-softmax
attention for its own rows against all 3072 columns.

Attention matmuls are "flipped": stationary = 128x128 block of the masked
exp matrix p (j on partitions), moving = gathered [ones|Wh] chunk, so the
output lands as [i, o] with the softmax denominator in column 0 --
normalization is a per-partition scalar multiply. ELU'd outputs accumulate
in SBUF in [i, o] layout and one PE-transpose pass per layer rebuilds the
h^T chunk tiles for the next layer's matmuls; h never round-trips DRAM.

Softmax uses the overflow-safe identity
    exp(lrelu(z)) = max(exp(z), exp(.02 z)),  z = s_i + d_j
with s, d falling out of the Wh matmul via fused weight columns
(W@a_dst, W@a_src appended to W).
"""

import os
import numpy as np

import concourse.bass as bass
import concourse.bacc as bacc
import concourse.mybir as mybir
import concourse.tile as tile
from concourse.bass_utils import run_bass_kernel_spmd

# ---- problem constants (hardcoded; kernel.py must be self-contained) ----
N = 3072
F_IN = 300
HID = 300
OUT_ATT = 600
HEADS = 10
N_GRAPHS = 96
MLP_HID = 600
NOUT = 768
ALPHA = 0.02

NCORES = 8
RPC = N // NCORES          # 384 rows (nodes) per core
NJT = RPC // 128           # 3 own-row tiles of 128
NCH = N // 128             # 24 column chunks of 128
GRP = 4                    # chunks per elementwise group
NSLOT = 16                 # graph slots per core for the pooling AllGather

W2 = HID + 2               # fused R columns: Wh(300) | d | s
GW = HID + 2               # gathered per-head width: ones | Wh(300) | d
SW = HID + 1               # streamed width: ones | Wh(300)
WO2 = OUT_ATT + 2          # fused Ro columns: Wh(600) | d | s
GWO = OUT_ATT + 2          # gathered: ones | Wh(600) | d
SWO = OUT_ATT + 1          # streamed: ones | Wh(600)

F32 = mybir.dt.float32
BF16 = mybir.dt.bfloat16
AF = mybir.ActivationFunctionType
CDT = BF16

TRACE = bool(os.environ.get("KERNEL_TRACE"))
DEBUG_STAGE = os.environ.get("KERNEL_DEBUG", "")

_compiled = {}


def _chunks(total, step=128):
    out = []
    lo = 0
    while lo < total:
        out.append((lo, min(step, total - lo)))
        lo += step
    return out


def _mm(nc, out, lhsT, rhs, **kw):
    if lhsT.dtype == F32:
        lhsT = lhsT.bitcast(mybir.dt.float32r)
        rhs = rhs.bitcast(mybir.dt.float32r)
    nc.tensor.matmul(out, lhsT, rhs, **kw)


def build():
    nc = bacc.Bacc("TRN2", target_bir_lowering=False, debug=False,
                   num_devices=NCORES)

    xT = nc.dram_tensor("xT", [F_IN, RPC], CDT, kind="ExternalInput")
    adjT = nc.dram_tensor("adjT", [N, RPC], CDT, kind="ExternalInput")
    smat16 = nc.dram_tensor("smat16", [RPC, NSLOT], CDT, kind="ExternalInput")
    cmat = nc.dram_tensor("cmat", [NCORES * NSLOT, N_GRAPHS], F32,
                          kind="ExternalInput")
    R0 = nc.dram_tensor("R0", [HEADS, F_IN, W2], CDT, kind="ExternalInput")
    R1 = nc.dram_tensor("R1", [HEADS, HEADS * HID, W2], CDT, kind="ExternalInput")
    Ro = nc.dram_tensor("Ro", [HEADS * HID, WO2], CDT, kind="ExternalInput")
    Wm1 = nc.dram_tensor("Wm1", [OUT_ATT, MLP_HID], CDT, kind="ExternalInput")
    bm1 = nc.dram_tensor("bm1", [MLP_HID, 1], F32, kind="ExternalInput")
    Wm2 = nc.dram_tensor("Wm2", [MLP_HID, NOUT], CDT, kind="ExternalInput")
    bm2 = nc.dram_tensor("bm2", [NOUT, 1], F32, kind="ExternalInput")
    eye128 = nc.dram_tensor("eye128", [128, 128], CDT, kind="ExternalInput")
    outT = nc.dram_tensor("outT", [NOUT, N_GRAPHS], F32, kind="ExternalOutput")
    dbg = None
    if DEBUG_STAGE in ("L0", "L1"):
        dbg = nc.dram_tensor("dbg", [HEADS * HID, RPC], F32, kind="ExternalOutput")
    elif DEBUG_STAGE == "HO":
        dbg = nc.dram_tensor("dbg", [RPC, OUT_ATT], F32, kind="ExternalOutput")
    elif DEBUG_STAGE == "POOL":
        dbg = nc.dram_tensor("dbg", [OUT_ATT, N_GRAPHS], F32, kind="ExternalOutput")

    rg = [list(range(NCORES))]
    FO = HEADS * HID
    kch_o = _chunks(FO)        # 23x128 + 56

    with tile.TileContext(nc) as tc:
        with (
            tc.tile_pool(name="persist", bufs=1) as persist,
            tc.tile_pool(name="whbuf", bufs=2) as whbufp,
            tc.tile_pool(name="rstream", bufs=4) as rstream,
            tc.tile_pool(name="ew", bufs=3) as ew,
            tc.tile_pool(name="small", bufs=2) as small,
            tc.tile_pool(name="ps", bufs=1, space="PSUM") as ps,
            tc.tile_pool(name="dram", bufs=1, space="DRAM") as dram,
        ):
            # ---------- persistent SBUF state ----------
            adj_sb = persist.tile([128, NCH * RPC], CDT, name="adj_sb")
            nc.sync.dma_start(adj_sb[:].rearrange("p (c i) -> p c i", i=RPC),
                              adjT[:].rearrange("(c p) i -> p c i", p=128))
            smat_sb = [persist.tile([128, NSLOT], CDT, name=f"smat{i}")
                       for i in range(NJT)]
            for i in range(NJT):
                nc.sync.dma_start(smat_sb[i][:],
                                  smat16[i * 128:(i + 1) * 128, :])
            cmat_sb = persist.tile([128, N_GRAPHS], F32, name="cmat_sb")
            nc.sync.dma_start(cmat_sb[:], cmat[:])
            eye_sb = persist.tile([128, 128], CDT, name="eye_sb")
            nc.sync.dma_start(eye_sb[:], eye128[:])
            onesf_sb = persist.tile([1, 128], F32, name="onesf_sb")
            nc.vector.memset(onesf_sb[:], 1.0)
            onesc_sb = persist.tile([1, 128], CDT, name="onesc_sb")
            nc.vector.memset(onesc_sb[:], 1.0)

            # MLP weights prefetched up-front on the scalar ring
            gsp = _chunks(OUT_ATT)
            msp = _chunks(MLP_HID)
            wm1_sb = []
            for g, (glo, gsz) in enumerate(gsp):
                w = persist.tile([128, MLP_HID], CDT, name=f"wm1_{g}")
                nc.scalar.dma_start(w[:gsz, :], Wm1[glo:glo + gsz, :])
                wm1_sb.append(w)
            wm2_sb = []
            for m, (mlo, msz) in enumerate(msp):
                w = persist.tile([128, NOUT], CDT, name=f"wm2_{m}")
                nc.scalar.dma_start(w[:msz, :], Wm2[mlo:mlo + msz, :])
                wm2_sb.append(w)
            bm1_sb = []
            for m, (mlo, msz) in enumerate(msp):
                b = persist.tile([128, 1], F32, name=f"bm1_{m}")
                nc.scalar.dma_start(b[:msz, :], bm1[mlo:mlo + msz, :])
                bm1_sb.append(b)

            # h^T chunk tiles (next-layer matmul inputs) + h in [i, o] layout
            ht = [persist.tile([128, RPC], CDT, name=f"ht{kc}")
                  for kc in range(len(kch_o))]
            for ci, (lo, sz) in enumerate(_chunks(F_IN)):
                nc.sync.dma_start(ht[ci][:sz, :], xT[lo:lo + sz, :])
            hfull = [persist.tile([128, FO], CDT, name=f"hfull{t}")
                     for t in range(NJT)]

            # broadcast tiles per head (held for the whole layer)
            sbc_all = persist.tile([128, HEADS * RPC], CDT, name="sbc_all")
            e02bc_all = persist.tile([128, HEADS * RPC], CDT, name="e02bc_all")

            def s_transpose_and_bcast(wl_sb, stride, hslot):
                """PE part of the per-head s handling: transpose the s
                columns (at offset stride-1 within each jt block of wl_sb)
                into a [1, RPC] row, then broadcast s and exp(.02 s)
                down 128 partitions into the *_all tiles at hslot."""
                pst = ps.tile([1, RPC], F32, tag="p6", name="pst")
                for jt in range(NJT):
                    base = jt * stride
                    nc.tensor.matmul(pst[:, jt * 128:(jt + 1) * 128],
                                     wl_sb[:, base + stride - 1:base + stride],
                                     eye_sb[:], start=True, stop=True)
                cs = slice(hslot * RPC, (hslot + 1) * RPC)
                s_row = small.tile([1, RPC], F32, tag="s_row")
                nc.scalar.activation(s_row[:], pst[:], AF.Copy)
                e2_row = small.tile([1, RPC], CDT, tag="e2_row")
                nc.scalar.activation(e2_row[:], pst[:], AF.Exp, scale=ALPHA)
                pb = ps.tile([128, RPC], F32, tag="p6", name="pb")
                nc.tensor.matmul(pb[:], onesf_sb[:], s_row[:],
                                 start=True, stop=True)
                nc.scalar.activation(sbc_all[:, cs], pb[:], AF.Copy)
                pb3 = ps.tile([128, RPC], F32, tag="p6", name="pb3")
                nc.tensor.matmul(pb3[:], onesc_sb[:], e2_row[:],
                                 start=True, stop=True)
                nc.scalar.activation(e02bc_all[:, cs], pb3[:], AF.Copy)

            # ============ one multi-head GAT layer ============
            def gat_layer(lidx, fin, r_dram, batches, pivot):
                kch = _chunks(fin)
                nkc = len(kch)
                wl = [dram.tile([RPC, bsz * GW], CDT, name=f"wl{lidx}_{b}")
                      for b, bsz in enumerate(batches)]
                wg = [dram.tile([N, bsz * GW], CDT, name=f"wg{lidx}_{b}",
                                addr_space="Shared")
                      for b, bsz in enumerate(batches)]
                hmap = []       # head -> (batch idx, offset, is batch end)
                for b, bsz in enumerate(batches):
                    for off in range(bsz):
                        hmap.append((b, off, off == bsz - 1))

                stride = W2 + 1        # per-jt block in wl_sb: ones|Wh|d|s
                state = {"pend": None}

                def wh_head(h):
                    b, h5, bend = hmap[h]
                    psw = [ps.tile([128, W2], F32, tag=f"p{jt}",
                                   name=f"psw{jt}") for jt in range(NJT)]
                    for ci, (lo, sz) in enumerate(kch):
                        r_t = rstream.tile([128, W2], CDT, tag="r", bufs=12)
                        nc.sync.dma_start(r_t[:sz, :], r_dram[h, lo:lo + sz, :])
                        for jt in range(NJT):
                            _mm(nc, psw[jt][:],
                                ht[ci][:sz, jt * 128:(jt + 1) * 128],
                                r_t[:sz, :],
                                start=(ci == 0), stop=(ci == nkc - 1))
                    # wl_sb per jt: [ones | Wh(300) | d | s]
                    wl_sb = small.tile([128, NJT * stride], CDT, tag="wl_sb")
                    for jt in range(NJT):
                        base = jt * stride
                        nc.vector.memset(wl_sb[:, base:base + 1], 1.0)
                        nc.scalar.activation(wl_sb[:, base + 1:base + stride],
                                             psw[jt][:], AF.Copy)
                        nc.sync.dma_start(
                            wl[b][jt * 128:(jt + 1) * 128,
                                  h5 * GW:(h5 + 1) * GW],
                            wl_sb[:, base:base + GW])
                    # defer the PE s-transpose/broadcast by one head so its
                    # ACT deps never bubble the PE queue
                    if state["pend"] is not None:
                        s_transpose_and_bcast(*state["pend"])
                    state["pend"] = (wl_sb, stride, h)
                    if bend:
                        nc.gpsimd.collective_compute(
                            "AllGather", mybir.AluOpType.bypass,
                            replica_groups=rg, ins=[wl[b].opt()],
                            outs=[wg[b].opt()])
                    if h == HEADS - 1:
                        s_transpose_and_bcast(*state["pend"])
                        state["pend"] = None

                def att_head(h):
                    b, h5, _ = hmap[h]
                    cs = slice(h * RPC, (h + 1) * RPC)
                    # PSUM banks: interleaved heads use p3-5 (psw owns p0-2);
                    # tail heads alternate p0-2/p3-5 so consecutive heads
                    # never wait on each other's drain
                    if pivot >= HEADS:
                        tg = [3, 4, 5] if h % 2 == 0 else [0, 1, 2]
                    elif h < pivot:
                        tg = [3, 4, 5]
                    else:
                        tg = [0, 1, 2] if (h - pivot) % 2 == 0 else [3, 4, 5]
                    whb = whbufp.tile([128, NCH * GW], CDT, tag="whb",
                                      name=f"whb{h}")
                    nc.sync.dma_start(
                        whb[:].rearrange("p (c w) -> p c w", w=GW),
                        wg[b][:, h5 * GW:(h5 + 1) * GW]
                        .rearrange("(c p) w -> p c w", p=128))
                    dcols = whb[:].rearrange("p (c w) -> p c w", w=GW)[:, :, GW - 1]
                    ed02 = small.tile([128, NCH], F32, tag="ed02",
                                      name=f"ed02_{h}")
                    nc.scalar.activation(ed02[:], dcols, AF.Exp, scale=ALPHA)

                    pa = [ps.tile([128, SW], F32, tag=f"p{tg[t]}",
                                  name=f"pa{t}") for t in range(NJT)]
                    for c0 in range(0, NCH, GRP):
                        g = c0 // GRP
                        meng = nc.vector if g % 3 == 2 else nc.gpsimd
                        a_t = ew.tile([128, GRP * RPC], CDT, tag="a", bufs=2)
                        b_t = ew.tile([128, GRP * RPC], CDT, tag="b", bufs=2)
                        for k in range(GRP):
                            c = c0 + k
                            ks = slice(k * RPC, (k + 1) * RPC)
                            nc.scalar.activation(
                                a_t[:, ks], sbc_all[:, cs], AF.Exp,
                                bias=whb[:, c * GW + GW - 1:c * GW + GW])
                            nc.vector.tensor_scalar_mul(
                                b_t[:, ks], e02bc_all[:, cs], ed02[:, c:c + 1])
                        m_t = ew.tile([128, GRP * RPC], CDT, tag="m", bufs=2)
                        nc.vector.tensor_max(m_t[:], a_t[:], b_t[:])
                        p_t = ew.tile([128, GRP * RPC], CDT, tag="p", bufs=3)
                        meng.tensor_mul(
                            p_t[:], m_t[:], adj_sb[:, c0 * RPC:(c0 + GRP) * RPC])
                        for k in range(GRP):
                            c = c0 + k
                            for t in range(NJT):
                                _mm(nc, pa[t][:],
                                    p_t[:, k * RPC + t * 128:k * RPC + (t + 1) * 128],
                                    whb[:, c * GW:c * GW + SW],
                                    start=(c == 0), stop=(c == NCH - 1))
                    # normalize + ELU straight into hfull; reciprocal reads
                    # the denominator column straight from PSUM (one DVE op)
                    for t in range(NJT):
                        rden = small.tile([128, 1], F32, tag="rden")
                        nc.vector.reciprocal(rden[:], pa[t][:, 0:1])
                        y = hfull[t][:, h * HID:(h + 1) * HID]
                        nc.scalar.mul(y, pa[t][:, 1:SW], rden[:])
                        q = ew.tile([128, HID], CDT, tag="q")
                        nc.scalar.activation(q[:], y, AF.Exp)
                        nc.gpsimd.tensor_scalar(q[:], q[:], -1.0, 0.0,
                                                mybir.AluOpType.add,
                                                mybir.AluOpType.min)
                        nc.vector.tensor_max(y, y, q[:])

                # software-pipelined: Wh of head pivot+h overlaps att of
                # head h, so the elementwise engines pace the whole layer
                # while PE stays busy
                def transpose_chunk(kc):
                    lo, sz = kch_o[kc]
                    psT = ps.tile([128, RPC], F32, tag=f"p{6 + kc % 2}",
                                  name=f"psT{kc}")
                    for t in range(NJT):
                        _mm(nc, psT[:sz, t * 128:(t + 1) * 128],
                            hfull[t][:, lo:lo + sz], eye_sb[:],
                            start=True, stop=True)
                    nc.scalar.activation(ht[kc][:sz, :], psT[:sz, :], AF.Copy)

                for h in range(min(pivot, HEADS)):
                    wh_head(h)
                for h in range(HEADS):
                    att_head(h)
                    if pivot + h < HEADS:
                        wh_head(pivot + h)

                # --- transpose h [i, o] -> h^T chunk tiles ---
                for kc in range(len(kch_o)):
                    transpose_chunk(kc)

            # ---------------- output attention layer + pool + MLP ----------
            def _tail():
                nkc = len(kch_o)
                wlo = dram.tile([RPC, GWO], CDT, name="wlo")
                wgo = dram.tile([N, GWO], CDT, name="wgo", addr_space="Shared")
                nsp = [(0, 512), (512, WO2 - 512)]
                stride = WO2 + 1       # ones|Wh(600)|d|s
                wlo_sb = small.tile([128, NJT * stride], CDT, tag="wlo_sb")
                for jt in range(NJT):
                    pswo = [ps.tile([128, sz], F32, tag=f"p{jt * 2 + si}",
                                    name=f"pswo{si}")
                            for si, (lo, sz) in enumerate(nsp)]
                    for ci, (lo, sz) in enumerate(kch_o):
                        r_t = rstream.tile([128, WO2], CDT, tag="ro", bufs=6)
                        nc.sync.dma_start(r_t[:sz, :], Ro[lo:lo + sz, :])
                        for si, (slo, ssz) in enumerate(nsp):
                            _mm(nc, pswo[si][:],
                                ht[ci][:sz, jt * 128:(jt + 1) * 128],
                                r_t[:sz, slo:slo + ssz],
                                start=(ci == 0), stop=(ci == nkc - 1))
                    base = jt * stride
                    nc.vector.memset(wlo_sb[:, base:base + 1], 1.0)
                    for si, (slo, ssz) in enumerate(nsp):
                        nc.scalar.activation(
                            wlo_sb[:, base + 1 + slo:base + 1 + slo + ssz],
                            pswo[si][:], AF.Copy)
                    nc.sync.dma_start(wlo[jt * 128:(jt + 1) * 128, :],
                                      wlo_sb[:, base:base + GWO])
                s_transpose_and_bcast(wlo_sb, stride, 0)
                nc.gpsimd.collective_compute(
                    "AllGather", mybir.AluOpType.bypass, replica_groups=rg,
                    ins=[wlo.opt()], outs=[wgo.opt()])

                cs = slice(0, RPC)
                onspl = [(0, 512), (512, SWO - 512)]   # slices of ones|Wh
                ps_o = [[ps.tile([128, sz], F32, tag=f"p{t * 2 + si}",
                                 name=f"pso{t}_{si}")
                         for si, (lo, sz) in enumerate(onspl)]
                        for t in range(NJT)]
                for c0 in range(0, NCH, GRP):
                    g = c0 // GRP
                    meng = nc.vector if g % 3 == 2 else nc.gpsimd
                    whcs = []
                    a_t = ew.tile([128, GRP * RPC], CDT, tag="a", bufs=2)
                    b_t = ew.tile([128, GRP * RPC], CDT, tag="b", bufs=2)
                    for k in range(GRP):
                        c = c0 + k
                        ks = slice(k * RPC, (k + 1) * RPC)
                        whc = rstream.tile([128, GWO], CDT, tag=f"whc{k}",
                                           name=f"whc{c}", bufs=2)
                        nc.scalar.dma_start(whc[:],
                                            wgo[c * 128:(c + 1) * 128, :])
                        whcs.append(whc)
                        ed02c = small.tile([128, 1], F32, tag="ed02c", bufs=3)
                        nc.scalar.activation(ed02c[:], whc[:, GWO - 1:GWO],
                                             AF.Exp, scale=ALPHA)
                        nc.scalar.activation(
                            a_t[:, ks], sbc_all[:, cs], AF.Exp,
                            bias=whc[:, GWO - 1:GWO])
                        nc.vector.tensor_scalar_mul(
                            b_t[:, ks], e02bc_all[:, cs], ed02c[:])
                    m_t = ew.tile([128, GRP * RPC], CDT, tag="m", bufs=2)
                    nc.vector.tensor_max(m_t[:], a_t[:], b_t[:])
                    p_t = ew.tile([128, GRP * RPC], CDT, tag="p", bufs=3)
                    meng.tensor_mul(p_t[:], m_t[:],
                                    adj_sb[:, c0 * RPC:(c0 + GRP) * RPC])
                    for k in range(GRP):
                        c = c0 + k
                        for t in range(NJT):
                            for si, (slo, ssz) in enumerate(onspl):
                                _mm(nc, ps_o[t][si][:],
                                    p_t[:, k * RPC + t * 128:k * RPC + (t + 1) * 128],
                                    whcs[k][:, slo:slo + ssz],
                                    start=(c == 0), stop=(c == NCH - 1))

                # normalize + ELU; att_tiles[t]: [128, 600]
                att_tiles = []
                for t in range(NJT):
                    rdeno = small.tile([128, 1], F32, tag="rdeno")
                    nc.vector.reciprocal(rdeno[:], ps_o[t][0][:, 0:1])
                    att_sb = ew.tile([128, OUT_ATT], CDT, tag=f"atts{t}",
                                     name=f"atts{t}", bufs=1)
                    nc.vector.tensor_scalar_mul(att_sb[:, 0:511],
                                                ps_o[t][0][:, 1:512], rdeno[:])
                    nc.vector.tensor_scalar_mul(att_sb[:, 511:OUT_ATT],
                                                ps_o[t][1][:, 0:SWO - 512],
                                                rdeno[:])
                    q = ew.tile([128, OUT_ATT], CDT, tag="qo")
                    nc.scalar.activation(q[:], att_sb[:], AF.Exp)
                    nc.gpsimd.tensor_scalar(q[:], q[:], -1.0, 0.0,
                                            mybir.AluOpType.add,
                                            mybir.AluOpType.min)
                    nc.vector.tensor_max(att_sb[:], att_sb[:], q[:])
                    if DEBUG_STAGE == "HO":
                        a32 = ew.tile([128, OUT_ATT], F32, tag="a32dbg")
                        nc.vector.tensor_copy(a32[:], att_sb[:])
                        nc.sync.dma_start(dbg[t * 128:(t + 1) * 128, :], a32[:])
                    att_tiles.append(att_sb)

                # per-core slot pools [NSLOT, 600] -> AllGather -> combine
                pool_l = dram.tile([NSLOT, OUT_ATT], F32, name="pool_l")
                pool_g = dram.tile([NCORES * NSLOT, OUT_ATT], F32,
                                   name="pool_g", addr_space="Shared")
                psl = [(0, 512), (512, OUT_ATT - 512)]
                pq_sb = small.tile([NSLOT, OUT_ATT], F32, tag="pq_sb")
                for si, (slo, ssz) in enumerate(psl):
                    psq = ps.tile([NSLOT, ssz], F32, tag=f"p{6 + si}",
                                  name=f"psq{si}")
                    for t in range(NJT):
                        _mm(nc, psq[:], smat_sb[t][:],
                            att_tiles[t][:, slo:slo + ssz],
                            start=(t == 0), stop=(t == NJT - 1))
                    nc.scalar.activation(pq_sb[:, slo:slo + ssz], psq[:],
                                         AF.Copy)
                nc.sync.dma_start(pool_l[:], pq_sb[:])
                nc.gpsimd.collective_compute(
                    "AllGather", mybir.AluOpType.bypass, replica_groups=rg,
                    ins=[pool_l.opt()], outs=[pool_g.opt()])

                # ---------------- MLP (replicated) ----------------
                pg16 = persist.tile([128, OUT_ATT], F32, name="pg16")
                nc.sync.dma_start(pg16[:], pool_g[:])
                pg_sb = []
                for g, (glo, gsz) in enumerate(gsp):
                    psm = ps.tile([128, N_GRAPHS], F32, tag=f"p{g % 2}",
                                  name=f"psg{g}")
                    nc.tensor.matmul(psm[:gsz, :], pg16[:, glo:glo + gsz],
                                     cmat_sb[:], start=True, stop=True)
                    t = persist.tile([128, N_GRAPHS], CDT, name=f"pg{g}")
                    nc.vector.tensor_copy(t[:gsz, :], psm[:gsz, :])
                    pg_sb.append(t)
                if DEBUG_STAGE == "POOL":
                    for g, (glo, gsz) in enumerate(gsp):
                        t32 = small.tile([128, N_GRAPHS], F32, tag="pooldbg")
                        nc.vector.tensor_copy(t32[:gsz, :], pg_sb[g][:gsz, :])
                        nc.sync.dma_start(dbg[glo:glo + gsz, :], t32[:gsz, :])
                h1_sb = []
                for m, (mlo, msz) in enumerate(msp):
                    psm = ps.tile([128, N_GRAPHS], F32, tag=f"p{m % 2}")
                    for g, (glo, gsz) in enumerate(gsp):
                        _mm(nc, psm[:msz, :], wm1_sb[g][:gsz, mlo:mlo + msz],
                            pg_sb[g][:gsz, :], start=(g == 0),
                            stop=(g == len(gsp) - 1))
                    t = persist.tile([128, N_GRAPHS], CDT, name=f"h1_{m}")
                    nc.scalar.activation(t[:msz, :], psm[:msz, :], AF.Relu,
                                         bias=bm1_sb[m][:msz, :])
                    h1_sb.append(t)
                for o, (olo, osz) in enumerate(_chunks(NOUT)):
                    b2 = small.tile([128, 1], F32, tag="bm2")
                    nc.sync.dma_start(b2[:osz, :], bm2[olo:olo + osz, :])
                    psm = ps.tile([128, N_GRAPHS], F32, tag=f"p{2 + o % 2}")
                    for m, (mlo, msz) in enumerate(msp):
                        _mm(nc, psm[:osz, :], wm2_sb[m][:msz, olo:olo + osz],
                            h1_sb[m][:msz, :], start=(m == 0),
                            stop=(m == len(msp) - 1))
                    ot = small.tile([128, N_GRAPHS], F32, tag="ot")
                    nc.vector.tensor_scalar_add(ot[:osz, :], psm[:osz, :],
                                                b2[:osz, :])
                    nc.sync.dma_start(outT[olo:olo + osz, :], ot[:osz, :])

            # ---------------- run the stages ----------------
            def dump_ht():
                for kc, (lo, sz) in enumerate(kch_o):
                    t32 = rstream.tile([128, RPC], F32, tag="tdb32",
                                       name=f"t32{kc}")
                    nc.vector.tensor_copy(t32[:sz, :], ht[kc][:sz, :])
                    nc.sync.dma_start(dbg[lo:lo + sz, :], t32[:sz, :])

            gat_layer(0, F_IN, R0, [1, 1, 2, 3, 3], 10)
            stop = False
            if DEBUG_STAGE == "L0":
                dump_ht()
                stop = True
            if not stop:
                gat_layer(1, FO, R1, [1, 1, 2, 3, 3], 5)
                if DEBUG_STAGE == "L1":
                    dump_ht()
                    stop = True
            if not stop:
                _tail()

    nc.compile()
    return nc


# ======================= host side =======================

def _np_cdt(a):
    import ml_dtypes
    return np.ascontiguousarray(np.asarray(a, np.float32).astype(ml_dtypes.bfloat16))


def _prep_inputs(x, edge_index, batch, W0, a0_src, a0_dst, W1, a1_src, a1_dst,
                 W_out, ao_src, ao_dst, Wm1, bm1, Wm2, bm2):
    x = np.asarray(x, np.float32)
    ei = np.asarray(edge_index)
    batch = np.asarray(batch).astype(np.int64)
    adj = np.zeros((N, N), np.float32)
    adj[ei[0], ei[1]] = 1.0

    cnt = np.bincount(batch, minlength=N_GRAPHS).astype(np.float32)
    cnt = np.maximum(cnt, 1.0)
    # per-core slot pooling: core c's rows span graphs [lo_c, lo_c+NSLOT)
    lo_cs, smat16s = [], []
    cmat_np = np.zeros((NCORES * NSLOT, N_GRAPHS), np.float32)
    for c in range(NCORES):
        b = batch[c * RPC:(c + 1) * RPC]
        lo = int(b.min())
        assert int(b.max()) - lo + 1 <= NSLOT, "graph span exceeds NSLOT"
        lo_cs.append(lo)
        sm = np.zeros((RPC, NSLOT), np.float32)
        sm[np.arange(RPC), b - lo] = 1.0 / cnt[b]
        smat16s.append(sm)
        for k in range(NSLOT):
            g = lo + k
            if g < N_GRAPHS:
                cmat_np[c * NSLOT + k, g] = 1.0

    W0 = np.asarray(W0, np.float32)
    W1 = np.asarray(W1, np.float32)
    W_out = np.asarray(W_out, np.float32)

    def fuse(W, a_dst, a_src):   # [H,F,O],[H,O],[H,O] -> [H,F,O+2]
        wad = np.einsum('hfo,ho->hf', W, np.asarray(a_dst, np.float32))
        was = np.einsum('hfo,ho->hf', W, np.asarray(a_src, np.float32))
        return np.concatenate([W, wad[:, :, None], was[:, :, None]], axis=2)

    R0p = fuse(W0, a0_dst, a0_src)
    R1p = fuse(W1, a1_dst, a1_src)
    Rop = np.concatenate(
        [W_out, (W_out @ np.asarray(ao_dst, np.float32))[:, None],
         (W_out @ np.asarray(ao_src, np.float32))[:, None]], axis=1)

    shared = dict(
        R0=_np_cdt(R0p), R1=_np_cdt(R1p), Ro=_np_cdt(Rop),
        Wm1=_np_cdt(Wm1),
        bm1=np.ascontiguousarray(np.asarray(bm1, np.float32)[:, None]),
        Wm2=_np_cdt(Wm2),
        bm2=np.ascontiguousarray(np.asarray(bm2, np.float32)[:, None]),
        eye128=_np_cdt(np.eye(128, dtype=np.float32)),
        cmat=np.ascontiguousarray(cmat_np),
    )
    xT_full = x.T
    in_maps = []
    for c in range(NCORES):
        rows = slice(c * RPC, (c + 1) * RPC)
        m = dict(shared)
        m["xT"] = _np_cdt(xT_full[:, rows])
        m["adjT"] = _np_cdt(adj[rows, :].T)
        m["smat16"] = _np_cdt(smat16s[c])
        in_maps.append(m)
    return in_maps


_last_results = None


def kernel(**inputs):
    global _last_results
    if "k" not in _compiled:
        _compiled["k"] = build()
    nc = _compiled["k"]
    in_maps = _prep_inputs(**inputs)
    kw = {}
    if TRACE:
        try:
            import tracehook
            tracehook.install()
            kw = dict(trace=True)
            td = os.environ.get("KERNEL_TRACEDIR")
            if td:
                kw["tmpdir"] = td
        except ImportError:
            pass
    res = run_bass_kernel_spmd(nc, in_maps, core_ids=list(range(NCORES)), **kw)
    _last_results = res
    return np.ascontiguousarray(res.results[0]["outT"].T)



# revision 47
# speedup vs baseline: 1.2264x; 1.2264x over previous
"""Trainium2 Bass kernel for nn_DeepGATEncoder (3-layer GAT + mean-pool + MLP).

Sharding: node rows split 384/core across 8 cores; weights replicated.
Per GAT layer each core computes Wh (+ fused a_dst / a_src columns) for its
own 384 nodes for all 10 heads; the per-head [ones|Wh|d] blocks are
AllGather'ed in batched collectives ([1,1,2,3,3] heads -- small first so
the first att head unblocks early), then each core runs masked-softmax
attention for its own rows against all 3072 columns.

Attention matmuls are "flipped": stationary = 128x128 block of the masked
exp matrix p (j on partitions), moving = gathered [ones|Wh] chunk, so the
output lands as [i, o] with the softmax denominator in column 0 --
normalization is a per-partition scalar multiply. ELU'd outputs accumulate
in SBUF in [i, o] layout and one PE-transpose pass per layer rebuilds the
h^T chunk tiles for the next layer's matmuls; h never round-trips DRAM.

Softmax uses the overflow-safe identity
    exp(lrelu(z)) = max(exp(z), exp(.02 z)),  z = s_i + d_j
with s, d falling out of the Wh matmul via fused weight columns
(W@a_dst, W@a_src appended to W).
"""

import os
import numpy as np

import concourse.bass as bass
import concourse.bacc as bacc
import concourse.mybir as mybir
import concourse.tile as tile
from concourse.bass_utils import run_bass_kernel_spmd

# ---- problem constants (hardcoded; kernel.py must be self-contained) ----
N = 3072
F_IN = 300
HID = 300
OUT_ATT = 600
HEADS = 10
N_GRAPHS = 96
MLP_HID = 600
NOUT = 768
ALPHA = 0.02

NCORES = 8
RPC = N // NCORES          # 384 rows (nodes) per core
NJT = RPC // 128           # 3 own-row tiles of 128
NCH = N // 128             # 24 column chunks of 128
GRP = 4                    # chunks per elementwise group
NSLOT = 16                 # graph slots per core for the pooling AllGather

W2 = HID + 2               # fused R columns: Wh(300) | d | s
GW = HID + 2               # gathered per-head width: ones | Wh(300) | d
SW = HID + 1               # streamed width: ones | Wh(300)
WO2 = OUT_ATT + 2          # fused Ro columns: Wh(600) | d | s
GWO = OUT_ATT + 2          # gathered: ones | Wh(600) | d
SWO = OUT_ATT + 1          # streamed: ones | Wh(600)

F32 = mybir.dt.float32
BF16 = mybir.dt.bfloat16
AF = mybir.ActivationFunctionType
CDT = BF16

TRACE = bool(os.environ.get("KERNEL_TRACE"))
DEBUG_STAGE = os.environ.get("KERNEL_DEBUG", "")

_compiled = {}


def _chunks(total, step=128):
    out = []
    lo = 0
    while lo < total:
        out.append((lo, min(step, total - lo)))
        lo += step
    return out


def _mm(nc, out, lhsT, rhs, **kw):
    if lhsT.dtype == F32:
        lhsT = lhsT.bitcast(mybir.dt.float32r)
        rhs = rhs.bitcast(mybir.dt.float32r)
    nc.tensor.matmul(out, lhsT, rhs, **kw)


def build():
    nc = bacc.Bacc("TRN2", target_bir_lowering=False, debug=False,
                   num_devices=NCORES)

    xT = nc.dram_tensor("xT", [F_IN, RPC], CDT, kind="ExternalInput")
    adjT = nc.dram_tensor("adjT", [N, RPC], CDT, kind="ExternalInput")
    smat16 = nc.dram_tensor("smat16", [RPC, NSLOT], CDT, kind="ExternalInput")
    cmat = nc.dram_tensor("cmat", [NCORES * NSLOT, N_GRAPHS], F32,
                          kind="ExternalInput")
    R0 = nc.dram_tensor("R0", [HEADS, F_IN, W2], CDT, kind="ExternalInput")
    R1 = nc.dram_tensor("R1", [HEADS, HEADS * HID, W2], CDT, kind="ExternalInput")
    Ro = nc.dram_tensor("Ro", [HEADS * HID, WO2], CDT, kind="ExternalInput")
    Wm1 = nc.dram_tensor("Wm1", [OUT_ATT, MLP_HID], CDT, kind="ExternalInput")
    bm1 = nc.dram_tensor("bm1", [MLP_HID, 1], F32, kind="ExternalInput")
    Wm2 = nc.dram_tensor("Wm2", [MLP_HID, NOUT], CDT, kind="ExternalInput")
    bm2 = nc.dram_tensor("bm2", [NOUT, 1], F32, kind="ExternalInput")
    eye128 = nc.dram_tensor("eye128", [128, 128], CDT, kind="ExternalInput")
    outT = nc.dram_tensor("outT", [NOUT, N_GRAPHS], F32, kind="ExternalOutput")
    dbg = None
    if DEBUG_STAGE in ("L0", "L1"):
        dbg = nc.dram_tensor("dbg", [HEADS * HID, RPC], F32, kind="ExternalOutput")
    elif DEBUG_STAGE == "HO":
        dbg = nc.dram_tensor("dbg", [RPC, OUT_ATT], F32, kind="ExternalOutput")
    elif DEBUG_STAGE == "POOL":
        dbg = nc.dram_tensor("dbg", [OUT_ATT, N_GRAPHS], F32, kind="ExternalOutput")

    rg = [list(range(NCORES))]
    FO = HEADS * HID
    kch_o = _chunks(FO)        # 23x128 + 56

    with tile.TileContext(nc) as tc:
        with (
            tc.tile_pool(name="persist", bufs=1) as persist,
            tc.tile_pool(name="whbuf", bufs=2) as whbufp,
            tc.tile_pool(name="rstream", bufs=4) as rstream,
            tc.tile_pool(name="ew", bufs=3) as ew,
            tc.tile_pool(name="small", bufs=2) as small,
            tc.tile_pool(name="ps", bufs=1, space="PSUM") as ps,
            tc.tile_pool(name="dram", bufs=1, space="DRAM") as dram,
        ):
            # ---------- persistent SBUF state ----------
            adj_sb = persist.tile([128, NCH * RPC], CDT, name="adj_sb")
            nc.sync.dma_start(adj_sb[:].rearrange("p (c i) -> p c i", i=RPC),
                              adjT[:].rearrange("(c p) i -> p c i", p=128))
            smat_sb = [persist.tile([128, NSLOT], CDT, name=f"smat{i}")
                       for i in range(NJT)]
            for i in range(NJT):
                nc.sync.dma_start(smat_sb[i][:],
                                  smat16[i * 128:(i + 1) * 128, :])
            cmat_sb = persist.tile([128, N_GRAPHS], F32, name="cmat_sb")
            nc.sync.dma_start(cmat_sb[:], cmat[:])
            eye_sb = persist.tile([128, 128], CDT, name="eye_sb")
            nc.sync.dma_start(eye_sb[:], eye128[:])
            onesf_sb = persist.tile([1, 128], F32, name="onesf_sb")
            nc.vector.memset(onesf_sb[:], 1.0)
            onesc_sb = persist.tile([1, 128], CDT, name="onesc_sb")
            nc.vector.memset(onesc_sb[:], 1.0)

            # MLP weights prefetched up-front on the scalar ring
            gsp = _chunks(OUT_ATT)
            msp = _chunks(MLP_HID)
            wm1_sb = []
            for g, (glo, gsz) in enumerate(gsp):
                w = persist.tile([128, MLP_HID], CDT, name=f"wm1_{g}")
                nc.scalar.dma_start(w[:gsz, :], Wm1[glo:glo + gsz, :])
                wm1_sb.append(w)
            wm2_sb = []
            for m, (mlo, msz) in enumerate(msp):
                w = persist.tile([128, NOUT], CDT, name=f"wm2_{m}")
                nc.scalar.dma_start(w[:msz, :], Wm2[mlo:mlo + msz, :])
                wm2_sb.append(w)
            bm1_sb = []
            for m, (mlo, msz) in enumerate(msp):
                b = persist.tile([128, 1], F32, name=f"bm1_{m}")
                nc.scalar.dma_start(b[:msz, :], bm1[mlo:mlo + msz, :])
                bm1_sb.append(b)

            # h^T chunk tiles (next-layer matmul inputs) + h in [i, o] layout
            ht = [persist.tile([128, RPC], CDT, name=f"ht{kc}")
                  for kc in range(len(kch_o))]
            for ci, (lo, sz) in enumerate(_chunks(F_IN)):
                nc.sync.dma_start(ht[ci][:sz, :], xT[lo:lo + sz, :])
            hfull = [persist.tile([128, FO], CDT, name=f"hfull{t}")
                     for t in range(NJT)]

            # broadcast tiles per head (held for the whole layer)
            sbc_all = persist.tile([128, HEADS * RPC], CDT, name="sbc_all")
            e02bc_all = persist.tile([128, HEADS * RPC], CDT, name="e02bc_all")

            def s_transpose_and_bcast(wl_sb, stride, hslot):
                """PE part of the per-head s handling: transpose the s
                columns (at offset stride-1 within each jt block of wl_sb)
                into a [1, RPC] row, then broadcast s and exp(.02 s)
                down 128 partitions into the *_all tiles at hslot."""
                pst = ps.tile([1, RPC], F32, tag="p6", name="pst")
                for jt in range(NJT):
                    base = jt * stride
                    nc.tensor.matmul(pst[:, jt * 128:(jt + 1) * 128],
                                     wl_sb[:, base + stride - 1:base + stride],
                                     eye_sb[:], start=True, stop=True)
                cs = slice(hslot * RPC, (hslot + 1) * RPC)
                s_row = small.tile([1, RPC], F32, tag="s_row")
                nc.scalar.activation(s_row[:], pst[:], AF.Copy)
                e2_row = small.tile([1, RPC], CDT, tag="e2_row")
                nc.scalar.activation(e2_row[:], pst[:], AF.Exp, scale=ALPHA)
                pb = ps.tile([128, RPC], F32, tag="p6", name="pb")
                nc.tensor.matmul(pb[:], onesf_sb[:], s_row[:],
                                 start=True, stop=True)
                nc.scalar.activation(sbc_all[:, cs], pb[:], AF.Copy)
                pb3 = ps.tile([128, RPC], F32, tag="p6", name="pb3")
                nc.tensor.matmul(pb3[:], onesc_sb[:], e2_row[:],
                                 start=True, stop=True)
                nc.scalar.activation(e02bc_all[:, cs], pb3[:], AF.Copy)

            # ============ one multi-head GAT layer ============
            def gat_layer(lidx, fin, r_dram, batches, pivot):
                kch = _chunks(fin)
                nkc = len(kch)
                wl = [dram.tile([RPC, bsz * GW], CDT, name=f"wl{lidx}_{b}")
                      for b, bsz in enumerate(batches)]
                wg = [dram.tile([N, bsz * GW], CDT, name=f"wg{lidx}_{b}",
                                addr_space="Shared")
                      for b, bsz in enumerate(batches)]
                hmap = []       # head -> (batch idx, offset, is batch end)
                for b, bsz in enumerate(batches):
                    for off in range(bsz):
                        hmap.append((b, off, off == bsz - 1))

                stride = W2 + 1        # per-jt block in wl_sb: ones|Wh|d|s
                state = {"pend": None}

                def wh_head(h):
                    b, h5, bend = hmap[h]
                    psw = [ps.tile([128, W2], F32, tag=f"p{jt}",
                                   name=f"psw{jt}") for jt in range(NJT)]
                    for ci, (lo, sz) in enumerate(kch):
                        r_t = rstream.tile([128, W2], CDT, tag="r", bufs=12)
                        nc.sync.dma_start(r_t[:sz, :], r_dram[h, lo:lo + sz, :])
                        for jt in range(NJT):
                            _mm(nc, psw[jt][:],
                                ht[ci][:sz, jt * 128:(jt + 1) * 128],
                                r_t[:sz, :],
                                start=(ci == 0), stop=(ci == nkc - 1))
                    # wl_sb per jt: [ones | Wh(300) | d | s]
                    wl_sb = small.tile([128, NJT * stride], CDT, tag="wl_sb")
                    for jt in range(NJT):
                        base = jt * stride
                        nc.vector.memset(wl_sb[:, base:base + 1], 1.0)
                        nc.scalar.activation(wl_sb[:, base + 1:base + stride],
                                             psw[jt][:], AF.Copy)
                        nc.sync.dma_start(
                            wl[b][jt * 128:(jt + 1) * 128,
                                  h5 * GW:(h5 + 1) * GW],
                            wl_sb[:, base:base + GW])
                    # defer the PE s-transpose/broadcast by one head so its
                    # ACT deps never bubble the PE queue
                    if state["pend"] is not None:
                        s_transpose_and_bcast(*state["pend"])
                    state["pend"] = (wl_sb, stride, h)
                    if bend:
                        nc.gpsimd.collective_compute(
                            "AllGather", mybir.AluOpType.bypass,
                            replica_groups=rg, ins=[wl[b].opt()],
                            outs=[wg[b].opt()])
                    if h == HEADS - 1:
                        s_transpose_and_bcast(*state["pend"])
                        state["pend"] = None

                def att_head(h):
                    b, h5, _ = hmap[h]
                    cs = slice(h * RPC, (h + 1) * RPC)
                    # PSUM banks: interleaved heads use p3-5 (psw owns p0-2);
                    # tail heads alternate p0-2/p3-5 so consecutive heads
                    # never wait on each other's drain
                    if pivot >= HEADS:
                        tg = [3, 4, 5] if h % 2 == 0 else [0, 1, 2]
                    elif h < pivot:
                        tg = [3, 4, 5]
                    else:
                        tg = [0, 1, 2] if (h - pivot) % 2 == 0 else [3, 4, 5]
                    whb = whbufp.tile([128, NCH * GW], CDT, tag="whb",
                                      name=f"whb{h}")
                    nc.sync.dma_start(
                        whb[:].rearrange("p (c w) -> p c w", w=GW),
                        wg[b][:, h5 * GW:(h5 + 1) * GW]
                        .rearrange("(c p) w -> p c w", p=128))
                    dcols = whb[:].rearrange("p (c w) -> p c w", w=GW)[:, :, GW - 1]
                    ed02 = small.tile([128, NCH], F32, tag="ed02",
                                      name=f"ed02_{h}")
                    nc.scalar.activation(ed02[:], dcols, AF.Exp, scale=ALPHA)

                    pa = [ps.tile([128, SW], F32, tag=f"p{tg[t]}",
                                  name=f"pa{t}") for t in range(NJT)]
                    for c0 in range(0, NCH, GRP):
                        g = c0 // GRP
                        meng = nc.vector if g % 3 == 2 else nc.gpsimd
                        a_t = ew.tile([128, GRP * RPC], CDT, tag="a", bufs=2)
                        b_t = ew.tile([128, GRP * RPC], CDT, tag="b", bufs=2)
                        for k in range(GRP):
                            c = c0 + k
                            ks = slice(k * RPC, (k + 1) * RPC)
                            nc.scalar.activation(
                                a_t[:, ks], sbc_all[:, cs], AF.Exp,
                                bias=whb[:, c * GW + GW - 1:c * GW + GW])
                            nc.vector.tensor_scalar_mul(
                                b_t[:, ks], e02bc_all[:, cs], ed02[:, c:c + 1])
                        m_t = ew.tile([128, GRP * RPC], CDT, tag="m", bufs=2)
                        nc.vector.tensor_max(m_t[:], a_t[:], b_t[:])
                        p_t = ew.tile([128, GRP * RPC], CDT, tag="p", bufs=3)
                        meng.tensor_mul(
                            p_t[:], m_t[:], adj_sb[:, c0 * RPC:(c0 + GRP) * RPC])
                        for k in range(GRP):
                            c = c0 + k
                            for t in range(NJT):
                                _mm(nc, pa[t][:],
                                    p_t[:, k * RPC + t * 128:k * RPC + (t + 1) * 128],
                                    whb[:, c * GW:c * GW + SW],
                                    start=(c == 0), stop=(c == NCH - 1))
                    # normalize + ELU straight into hfull; reciprocal reads
                    # the denominator column straight from PSUM (one DVE op)
                    for t in range(NJT):
                        rden = small.tile([128, 1], F32, tag="rden")
                        nc.vector.reciprocal(rden[:], pa[t][:, 0:1])
                        y = hfull[t][:, h * HID:(h + 1) * HID]
                        nc.scalar.mul(y, pa[t][:, 1:SW], rden[:])
                        q = ew.tile([128, HID], CDT, tag="q")
                        nc.scalar.activation(q[:], y, AF.Exp)
                        nc.vector.tensor_scalar(q[:], q[:], -1.0, 0.0,
                                                mybir.AluOpType.add,
                                                mybir.AluOpType.min)
                        nc.vector.tensor_max(y, y, q[:])

                # software-pipelined: Wh of head pivot+h overlaps att of
                # head h, so the elementwise engines pace the whole layer
                # while PE stays busy
                def transpose_chunk(kc):
                    lo, sz = kch_o[kc]
                    psT = ps.tile([128, RPC], F32, tag=f"p{6 + kc % 2}",
                                  name=f"psT{kc}")
                    for t in range(NJT):
                        _mm(nc, psT[:sz, t * 128:(t + 1) * 128],
                            hfull[t][:, lo:lo + sz], eye_sb[:],
                            start=True, stop=True)
                    nc.scalar.activation(ht[kc][:sz, :], psT[:sz, :], AF.Copy)

                for h in range(min(pivot, HEADS)):
                    wh_head(h)
                for h in range(HEADS):
                    att_head(h)
                    if pivot + h < HEADS:
                        wh_head(pivot + h)

                # --- transpose h [i, o] -> h^T chunk tiles ---
                for kc in range(len(kch_o)):
                    transpose_chunk(kc)

            # ---------------- output attention layer + pool + MLP ----------
            def _tail():
                nkc = len(kch_o)
                wlo = dram.tile([RPC, GWO], CDT, name="wlo")
                wgo = dram.tile([N, GWO], CDT, name="wgo", addr_space="Shared")
                nsp = [(0, 512), (512, WO2 - 512)]
                stride = WO2 + 1       # ones|Wh(600)|d|s
                wlo_sb = small.tile([128, NJT * stride], CDT, tag="wlo_sb")
                for jt in range(NJT):
                    pswo = [ps.tile([128, sz], F32, tag=f"p{jt * 2 + si}",
                                    name=f"pswo{si}")
                            for si, (lo, sz) in enumerate(nsp)]
                    for ci, (lo, sz) in enumerate(kch_o):
                        r_t = rstream.tile([128, WO2], CDT, tag="ro", bufs=6)
                        nc.sync.dma_start(r_t[:sz, :], Ro[lo:lo + sz, :])
                        for si, (slo, ssz) in enumerate(nsp):
                            _mm(nc, pswo[si][:],
                                ht[ci][:sz, jt * 128:(jt + 1) * 128],
                                r_t[:sz, slo:slo + ssz],
                                start=(ci == 0), stop=(ci == nkc - 1))
                    base = jt * stride
                    nc.vector.memset(wlo_sb[:, base:base + 1], 1.0)
                    for si, (slo, ssz) in enumerate(nsp):
                        nc.scalar.activation(
                            wlo_sb[:, base + 1 + slo:base + 1 + slo + ssz],
                            pswo[si][:], AF.Copy)
                    nc.sync.dma_start(wlo[jt * 128:(jt + 1) * 128, :],
                                      wlo_sb[:, base:base + GWO])
                s_transpose_and_bcast(wlo_sb, stride, 0)
                nc.gpsimd.collective_compute(
                    "AllGather", mybir.AluOpType.bypass, replica_groups=rg,
                    ins=[wlo.opt()], outs=[wgo.opt()])

                cs = slice(0, RPC)
                onspl = [(0, 512), (512, SWO - 512)]   # slices of ones|Wh
                ps_o = [[ps.tile([128, sz], F32, tag=f"p{t * 2 + si}",
                                 name=f"pso{t}_{si}")
                         for si, (lo, sz) in enumerate(onspl)]
                        for t in range(NJT)]
                for c0 in range(0, NCH, GRP):
                    g = c0 // GRP
                    meng = nc.vector if g % 3 == 2 else nc.gpsimd
                    whcs = []
                    a_t = ew.tile([128, GRP * RPC], CDT, tag="a", bufs=2)
                    b_t = ew.tile([128, GRP * RPC], CDT, tag="b", bufs=2)
                    for k in range(GRP):
                        c = c0 + k
                        ks = slice(k * RPC, (k + 1) * RPC)
                        whc = rstream.tile([128, GWO], CDT, tag=f"whc{k}",
                                           name=f"whc{c}", bufs=2)
                        nc.scalar.dma_start(whc[:],
                                            wgo[c * 128:(c + 1) * 128, :])
                        whcs.append(whc)
                        ed02c = small.tile([128, 1], F32, tag="ed02c", bufs=3)
                        nc.scalar.activation(ed02c[:], whc[:, GWO - 1:GWO],
                                             AF.Exp, scale=ALPHA)
                        nc.scalar.activation(
                            a_t[:, ks], sbc_all[:, cs], AF.Exp,
                            bias=whc[:, GWO - 1:GWO])
                        nc.vector.tensor_scalar_mul(
                            b_t[:, ks], e02bc_all[:, cs], ed02c[:])
                    m_t = ew.tile([128, GRP * RPC], CDT, tag="m", bufs=2)
                    nc.vector.tensor_max(m_t[:], a_t[:], b_t[:])
                    p_t = ew.tile([128, GRP * RPC], CDT, tag="p", bufs=3)
                    meng.tensor_mul(p_t[:], m_t[:],
                                    adj_sb[:, c0 * RPC:(c0 + GRP) * RPC])
                    for k in range(GRP):
                        c = c0 + k
                        for t in range(NJT):
                            for si, (slo, ssz) in enumerate(onspl):
                                _mm(nc, ps_o[t][si][:],
                                    p_t[:, k * RPC + t * 128:k * RPC + (t + 1) * 128],
                                    whcs[k][:, slo:slo + ssz],
                                    start=(c == 0), stop=(c == NCH - 1))

                # normalize + ELU; att_tiles[t]: [128, 600]
                att_tiles = []
                for t in range(NJT):
                    rdeno = small.tile([128, 1], F32, tag="rdeno")
                    nc.vector.reciprocal(rdeno[:], ps_o[t][0][:, 0:1])
                    att_sb = ew.tile([128, OUT_ATT], CDT, tag=f"atts{t}",
                                     name=f"atts{t}", bufs=1)
                    nc.vector.tensor_scalar_mul(att_sb[:, 0:511],
                                                ps_o[t][0][:, 1:512], rdeno[:])
                    nc.vector.tensor_scalar_mul(att_sb[:, 511:OUT_ATT],
                                                ps_o[t][1][:, 0:SWO - 512],
                                                rdeno[:])
                    q = ew.tile([128, OUT_ATT], CDT, tag="qo")
                    nc.scalar.activation(q[:], att_sb[:], AF.Exp)
                    nc.vector.tensor_scalar(q[:], q[:], -1.0, 0.0,
                                            mybir.AluOpType.add,
                                            mybir.AluOpType.min)
                    nc.vector.tensor_max(att_sb[:], att_sb[:], q[:])
                    if DEBUG_STAGE == "HO":
                        a32 = ew.tile([128, OUT_ATT], F32, tag="a32dbg")
                        nc.vector.tensor_copy(a32[:], att_sb[:])
                        nc.sync.dma_start(dbg[t * 128:(t + 1) * 128, :], a32[:])
                    att_tiles.append(att_sb)

                # per-core slot pools [NSLOT, 600] -> AllGather -> combine
                pool_l = dram.tile([NSLOT, OUT_ATT], F32, name="pool_l")
                pool_g = dram.tile([NCORES * NSLOT, OUT_ATT], F32,
                                   name="pool_g", addr_space="Shared")
                psl = [(0, 512), (512, OUT_ATT - 512)]
                pq_sb = small.tile([NSLOT, OUT_ATT], F32, tag="pq_sb")
                for si, (slo, ssz) in enumerate(psl):
                    psq = ps.tile([NSLOT, ssz], F32, tag=f"p{6 + si}",
                                  name=f"psq{si}")
                    for t in range(NJT):
                        _mm(nc, psq[:], smat_sb[t][:],
                            att_tiles[t][:, slo:slo + ssz],
                            start=(t == 0), stop=(t == NJT - 1))
                    nc.scalar.activation(pq_sb[:, slo:slo + ssz], psq[:],
                                         AF.Copy)
                nc.sync.dma_start(pool_l[:], pq_sb[:])
                nc.gpsimd.collective_compute(
                    "AllGather", mybir.AluOpType.bypass, replica_groups=rg,
                    ins=[pool_l.opt()], outs=[pool_g.opt()])

                # ---------------- MLP (replicated) ----------------
                pg16 = persist.tile([128, OUT_ATT], F32, name="pg16")
                nc.sync.dma_start(pg16[:], pool_g[:])
                pg_sb = []
                for g, (glo, gsz) in enumerate(gsp):
                    psm = ps.tile([128, N_GRAPHS], F32, tag=f"p{g % 2}",
                                  name=f"psg{g}")
                    nc.tensor.matmul(psm[:gsz, :], pg16[:, glo:glo + gsz],
                                     cmat_sb[:], start=True, stop=True)
                    t = persist.tile([128, N_GRAPHS], CDT, name=f"pg{g}")
                    nc.vector.tensor_copy(t[:gsz, :], psm[:gsz, :])
                    pg_sb.append(t)
                if DEBUG_STAGE == "POOL":
                    for g, (glo, gsz) in enumerate(gsp):
                        t32 = small.tile([128, N_GRAPHS], F32, tag="pooldbg")
                        nc.vector.tensor_copy(t32[:gsz, :], pg_sb[g][:gsz, :])
                        nc.sync.dma_start(dbg[glo:glo + gsz, :], t32[:gsz, :])
                h1_sb = []
                for m, (mlo, msz) in enumerate(msp):
                    psm = ps.tile([128, N_GRAPHS], F32, tag=f"p{m % 2}")
                    for g, (glo, gsz) in enumerate(gsp):
                        _mm(nc, psm[:msz, :], wm1_sb[g][:gsz, mlo:mlo + msz],
                            pg_sb[g][:gsz, :], start=(g == 0),
                            stop=(g == len(gsp) - 1))
                    t = persist.tile([128, N_GRAPHS], CDT, name=f"h1_{m}")
                    nc.scalar.activation(t[:msz, :], psm[:msz, :], AF.Relu,
                                         bias=bm1_sb[m][:msz, :])
                    h1_sb.append(t)
                for o, (olo, osz) in enumerate(_chunks(NOUT)):
                    b2 = small.tile([128, 1], F32, tag="bm2")
                    nc.sync.dma_start(b2[:osz, :], bm2[olo:olo + osz, :])
                    psm = ps.tile([128, N_GRAPHS], F32, tag=f"p{2 + o % 2}")
                    for m, (mlo, msz) in enumerate(msp):
                        _mm(nc, psm[:osz, :], wm2_sb[m][:msz, olo:olo + osz],
                            h1_sb[m][:msz, :], start=(m == 0),
                            stop=(m == len(msp) - 1))
                    ot = small.tile([128, N_GRAPHS], F32, tag="ot")
                    nc.vector.tensor_scalar_add(ot[:osz, :], psm[:osz, :],
                                                b2[:osz, :])
                    nc.sync.dma_start(outT[olo:olo + osz, :], ot[:osz, :])

            # ---------------- run the stages ----------------
            def dump_ht():
                for kc, (lo, sz) in enumerate(kch_o):
                    t32 = rstream.tile([128, RPC], F32, tag="tdb32",
                                       name=f"t32{kc}")
                    nc.vector.tensor_copy(t32[:sz, :], ht[kc][:sz, :])
                    nc.sync.dma_start(dbg[lo:lo + sz, :], t32[:sz, :])

            gat_layer(0, F_IN, R0, [1, 1, 2, 3, 3], 10)
            stop = False
            if DEBUG_STAGE == "L0":
                dump_ht()
                stop = True
            if not stop:
                gat_layer(1, FO, R1, [1, 1, 2, 3, 3], 5)
                if DEBUG_STAGE == "L1":
                    dump_ht()
                    stop = True
            if not stop:
                _tail()

    nc.compile()
    return nc


# ======================= host side =======================

def _np_cdt(a):
    import ml_dtypes
    return np.ascontiguousarray(np.asarray(a, np.float32).astype(ml_dtypes.bfloat16))


def _prep_inputs(x, edge_index, batch, W0, a0_src, a0_dst, W1, a1_src, a1_dst,
                 W_out, ao_src, ao_dst, Wm1, bm1, Wm2, bm2):
    x = np.asarray(x, np.float32)
    ei = np.asarray(edge_index)
    batch = np.asarray(batch).astype(np.int64)
    adj = np.zeros((N, N), np.float32)
    adj[ei[0], ei[1]] = 1.0

    cnt = np.bincount(batch, minlength=N_GRAPHS).astype(np.float32)
    cnt = np.maximum(cnt, 1.0)
    # per-core slot pooling: core c's rows span graphs [lo_c, lo_c+NSLOT)
    lo_cs, smat16s = [], []
    cmat_np = np.zeros((NCORES * NSLOT, N_GRAPHS), np.float32)
    for c in range(NCORES):
        b = batch[c * RPC:(c + 1) * RPC]
        lo = int(b.min())
        assert int(b.max()) - lo + 1 <= NSLOT, "graph span exceeds NSLOT"
        lo_cs.append(lo)
        sm = np.zeros((RPC, NSLOT), np.float32)
        sm[np.arange(RPC), b - lo] = 1.0 / cnt[b]
        smat16s.append(sm)
        for k in range(NSLOT):
            g = lo + k
            if g < N_GRAPHS:
                cmat_np[c * NSLOT + k, g] = 1.0

    W0 = np.asarray(W0, np.float32)
    W1 = np.asarray(W1, np.float32)
    W_out = np.asarray(W_out, np.float32)

    def fuse(W, a_dst, a_src):   # [H,F,O],[H,O],[H,O] -> [H,F,O+2]
        wad = np.einsum('hfo,ho->hf', W, np.asarray(a_dst, np.float32))
        was = np.einsum('hfo,ho->hf', W, np.asarray(a_src, np.float32))
        return np.concatenate([W, wad[:, :, None], was[:, :, None]], axis=2)

    R0p = fuse(W0, a0_dst, a0_src)
    R1p = fuse(W1, a1_dst, a1_src)
    Rop = np.concatenate(
        [W_out, (W_out @ np.asarray(ao_dst, np.float32))[:, None],
         (W_out @ np.asarray(ao_src, np.float32))[:, None]], axis=1)

    shared = dict(
        R0=_np_cdt(R0p), R1=_np_cdt(R1p), Ro=_np_cdt(Rop),
        Wm1=_np_cdt(Wm1),
        bm1=np.ascontiguousarray(np.asarray(bm1, np.float32)[:, None]),
        Wm2=_np_cdt(Wm2),
        bm2=np.ascontiguousarray(np.asarray(bm2, np.float32)[:, None]),
        eye128=_np_cdt(np.eye(128, dtype=np.float32)),
        cmat=np.ascontiguousarray(cmat_np),
    )
    xT_full = x.T
    in_maps = []
    for c in range(NCORES):
        rows = slice(c * RPC, (c + 1) * RPC)
        m = dict(shared)
        m["xT"] = _np_cdt(xT_full[:, rows])
        m["adjT"] = _np_cdt(adj[rows, :].T)
        m["smat16"] = _np_cdt(smat16s[c])
        in_maps.append(m)
    return in_maps


_last_results = None


def kernel(**inputs):
    global _last_results
    if "k" not in _compiled:
        _compiled["k"] = build()
    nc = _compiled["k"]
    in_maps = _prep_inputs(**inputs)
    kw = {}
    if TRACE:
        try:
            import tracehook
            tracehook.install()
            kw = dict(trace=True)
            td = os.environ.get("KERNEL_TRACEDIR")
            if td:
                kw["tmpdir"] = td
        except ImportError:
            pass
    res = run_bass_kernel_spmd(nc, in_maps, core_ids=list(range(NCORES)), **kw)
    _last_results = res
    return np.ascontiguousarray(res.results[0]["outT"].T)



# revision 48
# speedup vs baseline: 1.2612x; 1.0284x over previous
"""Trainium2 Bass kernel for nn_DeepGATEncoder (3-layer GAT + mean-pool + MLP).

Sharding: node rows split 384/core across 8 cores; weights replicated.
Per GAT layer each core computes Wh (+ fused a_dst / a_src columns) for its
own 384 nodes for all 10 heads; the per-head [ones|Wh|d] blocks are
AllGather'ed in batched collectives ([1,1,2,3,3] heads -- small first so
the first att head unblocks early), then each core runs masked-softmax
attention for its own rows against all 3072 columns.

Attention matmuls are "flipped": stationary = 128x128 block of the masked
exp matrix p (j on partitions), moving = gathered [ones|Wh] chunk, so the
output lands as [i, o] with the softmax denominator in column 0 --
normalization is a per-partition scalar multiply. ELU'd outputs accumulate
in SBUF in [i, o] layout and one PE-transpose pass per layer rebuilds the
h^T chunk tiles for the next layer's matmuls; h never round-trips DRAM.

Softmax uses the overflow-safe identity
    exp(lrelu(z)) = max(exp(z), exp(.02 z)),  z = s_i + d_j
with s, d falling out of the Wh matmul via fused weight columns
(W@a_dst, W@a_src appended to W).
"""

import os
import numpy as np

import concourse.bass as bass
import concourse.bacc as bacc
import concourse.mybir as mybir
import concourse.tile as tile
from concourse.bass_utils import run_bass_kernel_spmd

# ---- problem constants (hardcoded; kernel.py must be self-contained) ----
N = 3072
F_IN = 300
HID = 300
OUT_ATT = 600
HEADS = 10
N_GRAPHS = 96
MLP_HID = 600
NOUT = 768
ALPHA = 0.02

NCORES = 8
RPC = N // NCORES          # 384 rows (nodes) per core
NJT = RPC // 128           # 3 own-row tiles of 128
NCH = N // 128             # 24 column chunks of 128
GRP = 4                    # chunks per elementwise group
NSLOT = 16                 # graph slots per core for the pooling AllGather

W2 = HID + 2               # fused R columns: Wh(300) | d | s
GW = HID + 2               # gathered per-head width: ones | Wh(300) | d
SW = HID + 1               # streamed width: ones | Wh(300)
WO2 = OUT_ATT + 2          # fused Ro columns: Wh(600) | d | s
GWO = OUT_ATT + 2          # gathered: ones | Wh(600) | d
SWO = OUT_ATT + 1          # streamed: ones | Wh(600)

F32 = mybir.dt.float32
BF16 = mybir.dt.bfloat16
AF = mybir.ActivationFunctionType
CDT = BF16

TRACE = bool(os.environ.get("KERNEL_TRACE"))
DEBUG_STAGE = os.environ.get("KERNEL_DEBUG", "")

_compiled = {}


def _chunks(total, step=128):
    out = []
    lo = 0
    while lo < total:
        out.append((lo, min(step, total - lo)))
        lo += step
    return out


def _mm(nc, out, lhsT, rhs, **kw):
    if lhsT.dtype == F32:
        lhsT = lhsT.bitcast(mybir.dt.float32r)
        rhs = rhs.bitcast(mybir.dt.float32r)
    nc.tensor.matmul(out, lhsT, rhs, **kw)


def build():
    nc = bacc.Bacc("TRN2", target_bir_lowering=False, debug=False,
                   num_devices=NCORES)

    xT = nc.dram_tensor("xT", [F_IN, RPC], CDT, kind="ExternalInput")
    adjT = nc.dram_tensor("adjT", [N, RPC], CDT, kind="ExternalInput")
    smat16 = nc.dram_tensor("smat16", [RPC, NSLOT], CDT, kind="ExternalInput")
    cmat = nc.dram_tensor("cmat", [NCORES * NSLOT, N_GRAPHS], F32,
                          kind="ExternalInput")
    R0 = nc.dram_tensor("R0", [HEADS, F_IN, W2], CDT, kind="ExternalInput")
    R1 = nc.dram_tensor("R1", [HEADS, HEADS * HID, W2], CDT, kind="ExternalInput")
    Ro = nc.dram_tensor("Ro", [HEADS * HID, WO2], CDT, kind="ExternalInput")
    Wm1 = nc.dram_tensor("Wm1", [OUT_ATT, MLP_HID], CDT, kind="ExternalInput")
    bm1 = nc.dram_tensor("bm1", [MLP_HID, 1], F32, kind="ExternalInput")
    Wm2 = nc.dram_tensor("Wm2", [MLP_HID, NOUT], CDT, kind="ExternalInput")
    bm2 = nc.dram_tensor("bm2", [NOUT, 1], F32, kind="ExternalInput")
    eye128 = nc.dram_tensor("eye128", [128, 128], CDT, kind="ExternalInput")
    outT = nc.dram_tensor("outT", [NOUT, N_GRAPHS], F32, kind="ExternalOutput")
    dbg = None
    if DEBUG_STAGE in ("L0", "L1"):
        dbg = nc.dram_tensor("dbg", [HEADS * HID, RPC], F32, kind="ExternalOutput")
    elif DEBUG_STAGE == "HO":
        dbg = nc.dram_tensor("dbg", [RPC, OUT_ATT], F32, kind="ExternalOutput")
    elif DEBUG_STAGE == "POOL":
        dbg = nc.dram_tensor("dbg", [OUT_ATT, N_GRAPHS], F32, kind="ExternalOutput")

    rg = [list(range(NCORES))]
    FO = HEADS * HID
    kch_o = _chunks(FO)        # 23x128 + 56

    with tile.TileContext(nc) as tc:
        with (
            tc.tile_pool(name="persist", bufs=1) as persist,
            tc.tile_pool(name="whbuf", bufs=2) as whbufp,
            tc.tile_pool(name="rstream", bufs=4) as rstream,
            tc.tile_pool(name="ew", bufs=3) as ew,
            tc.tile_pool(name="small", bufs=2) as small,
            tc.tile_pool(name="ps", bufs=1, space="PSUM") as ps,
            tc.tile_pool(name="dram", bufs=1, space="DRAM") as dram,
        ):
            # ---------- persistent SBUF state ----------
            adj_sb = persist.tile([128, NCH * RPC], CDT, name="adj_sb")
            nc.sync.dma_start(adj_sb[:].rearrange("p (c i) -> p c i", i=RPC),
                              adjT[:].rearrange("(c p) i -> p c i", p=128))
            smat_sb = [persist.tile([128, NSLOT], CDT, name=f"smat{i}")
                       for i in range(NJT)]
            for i in range(NJT):
                nc.sync.dma_start(smat_sb[i][:],
                                  smat16[i * 128:(i + 1) * 128, :])
            cmat_sb = persist.tile([128, N_GRAPHS], F32, name="cmat_sb")
            nc.sync.dma_start(cmat_sb[:], cmat[:])
            eye_sb = persist.tile([128, 128], CDT, name="eye_sb")
            nc.sync.dma_start(eye_sb[:], eye128[:])
            onesf_sb = persist.tile([1, 128], F32, name="onesf_sb")
            nc.vector.memset(onesf_sb[:], 1.0)
            onesc_sb = persist.tile([1, 128], CDT, name="onesc_sb")
            nc.vector.memset(onesc_sb[:], 1.0)

            # MLP weights prefetched up-front on the scalar ring
            gsp = _chunks(OUT_ATT)
            msp = _chunks(MLP_HID)
            wm1_sb = []
            for g, (glo, gsz) in enumerate(gsp):
                w = persist.tile([128, MLP_HID], CDT, name=f"wm1_{g}")
                nc.scalar.dma_start(w[:gsz, :], Wm1[glo:glo + gsz, :])
                wm1_sb.append(w)
            wm2_sb = []
            for m, (mlo, msz) in enumerate(msp):
                w = persist.tile([128, NOUT], CDT, name=f"wm2_{m}")
                nc.scalar.dma_start(w[:msz, :], Wm2[mlo:mlo + msz, :])
                wm2_sb.append(w)
            bm1_sb = []
            for m, (mlo, msz) in enumerate(msp):
                b = persist.tile([128, 1], F32, name=f"bm1_{m}")
                nc.scalar.dma_start(b[:msz, :], bm1[mlo:mlo + msz, :])
                bm1_sb.append(b)

            # h^T chunk tiles (next-layer matmul inputs) + h in [i, o] layout
            ht = [persist.tile([128, RPC], CDT, name=f"ht{kc}")
                  for kc in range(len(kch_o))]
            for ci, (lo, sz) in enumerate(_chunks(F_IN)):
                nc.sync.dma_start(ht[ci][:sz, :], xT[lo:lo + sz, :])
            hfull = [persist.tile([128, FO], CDT, name=f"hfull{t}")
                     for t in range(NJT)]

            # broadcast tiles per head (held for the whole layer)
            sbc_all = persist.tile([128, HEADS * RPC], CDT, name="sbc_all")
            e02bc_all = persist.tile([128, HEADS * RPC], CDT, name="e02bc_all")

            def s_transpose_and_bcast(wl_sb, stride, hslot):
                """PE part of the per-head s handling: transpose the s
                columns (at offset stride-1 within each jt block of wl_sb)
                into a [1, RPC] row, then broadcast s and exp(.02 s)
                down 128 partitions into the *_all tiles at hslot."""
                pst = ps.tile([1, RPC], F32, tag="p6", name="pst")
                for jt in range(NJT):
                    base = jt * stride
                    nc.tensor.matmul(pst[:, jt * 128:(jt + 1) * 128],
                                     wl_sb[:, base + stride - 1:base + stride],
                                     eye_sb[:], start=True, stop=True)
                cs = slice(hslot * RPC, (hslot + 1) * RPC)
                s_row = small.tile([1, RPC], F32, tag="s_row")
                nc.scalar.activation(s_row[:], pst[:], AF.Copy)
                e2_row = small.tile([1, RPC], CDT, tag="e2_row")
                nc.scalar.activation(e2_row[:], pst[:], AF.Exp, scale=ALPHA)
                pb = ps.tile([128, RPC], F32, tag="p6", name="pb")
                nc.tensor.matmul(pb[:], onesf_sb[:], s_row[:],
                                 start=True, stop=True)
                nc.scalar.activation(sbc_all[:, cs], pb[:], AF.Copy)
                pb3 = ps.tile([128, RPC], F32, tag="p6", name="pb3")
                nc.tensor.matmul(pb3[:], onesc_sb[:], e2_row[:],
                                 start=True, stop=True)
                nc.scalar.activation(e02bc_all[:, cs], pb3[:], AF.Copy)

            # ============ one multi-head GAT layer ============
            def gat_layer(lidx, fin, r_dram, batches, pivot):
                kch = _chunks(fin)
                nkc = len(kch)
                wl = [dram.tile([RPC, bsz * GW], CDT, name=f"wl{lidx}_{b}")
                      for b, bsz in enumerate(batches)]
                wg = [dram.tile([N, bsz * GW], CDT, name=f"wg{lidx}_{b}",
                                addr_space="Shared")
                      for b, bsz in enumerate(batches)]
                hmap = []       # head -> (batch idx, offset, is batch end)
                for b, bsz in enumerate(batches):
                    for off in range(bsz):
                        hmap.append((b, off, off == bsz - 1))

                stride = W2 + 1        # per-jt block in wl_sb: ones|Wh|d|s
                state = {"pend": None}

                def wh_head(h):
                    b, h5, bend = hmap[h]
                    psw = [ps.tile([128, W2], F32, tag=f"p{jt}",
                                   name=f"psw{jt}") for jt in range(NJT)]
                    for ci, (lo, sz) in enumerate(kch):
                        r_t = rstream.tile([128, W2], CDT, tag="r", bufs=12)
                        nc.sync.dma_start(r_t[:sz, :], r_dram[h, lo:lo + sz, :])
                        for jt in range(NJT):
                            _mm(nc, psw[jt][:],
                                ht[ci][:sz, jt * 128:(jt + 1) * 128],
                                r_t[:sz, :],
                                start=(ci == 0), stop=(ci == nkc - 1))
                    # wl_sb per jt: [ones | Wh(300) | d | s]
                    wl_sb = small.tile([128, NJT * stride], CDT, tag="wl_sb")
                    for jt in range(NJT):
                        base = jt * stride
                        nc.vector.memset(wl_sb[:, base:base + 1], 1.0)
                        nc.scalar.activation(wl_sb[:, base + 1:base + stride],
                                             psw[jt][:], AF.Copy)
                        nc.sync.dma_start(
                            wl[b][jt * 128:(jt + 1) * 128,
                                  h5 * GW:(h5 + 1) * GW],
                            wl_sb[:, base:base + GW])
                    # defer the PE s-transpose/broadcast by one head so its
                    # ACT deps never bubble the PE queue
                    if state["pend"] is not None:
                        s_transpose_and_bcast(*state["pend"])
                    state["pend"] = (wl_sb, stride, h)
                    if bend:
                        nc.gpsimd.collective_compute(
                            "AllGather", mybir.AluOpType.bypass,
                            replica_groups=rg, ins=[wl[b].opt()],
                            outs=[wg[b].opt()])
                    if h == HEADS - 1:
                        s_transpose_and_bcast(*state["pend"])
                        state["pend"] = None

                def att_head(h):
                    b, h5, _ = hmap[h]
                    cs = slice(h * RPC, (h + 1) * RPC)
                    # PSUM banks: interleaved heads use p3-5 (psw owns p0-2);
                    # tail heads alternate p0-2/p3-5 so consecutive heads
                    # never wait on each other's drain
                    if pivot >= HEADS:
                        tg = [3, 4, 5] if h % 2 == 0 else [0, 1, 2]
                    elif h < pivot:
                        tg = [3, 4, 5]
                    else:
                        tg = [0, 1, 2] if (h - pivot) % 2 == 0 else [3, 4, 5]
                    whb = whbufp.tile([128, NCH * GW], CDT, tag="whb",
                                      name=f"whb{h}")
                    nc.sync.dma_start(
                        whb[:].rearrange("p (c w) -> p c w", w=GW),
                        wg[b][:, h5 * GW:(h5 + 1) * GW]
                        .rearrange("(c p) w -> p c w", p=128))
                    dcols = whb[:].rearrange("p (c w) -> p c w", w=GW)[:, :, GW - 1]
                    ed02 = small.tile([128, NCH], F32, tag="ed02",
                                      name=f"ed02_{h}")
                    nc.scalar.activation(ed02[:], dcols, AF.Exp, scale=ALPHA)

                    pa = [ps.tile([128, SW], F32, tag=f"p{tg[t]}",
                                  name=f"pa{t}") for t in range(NJT)]
                    for c0 in range(0, NCH, GRP):
                        g = c0 // GRP
                        meng = nc.vector if g % 3 == 2 else nc.gpsimd
                        a_t = ew.tile([128, GRP * RPC], CDT, tag="a", bufs=2)
                        b_t = ew.tile([128, GRP * RPC], CDT, tag="b", bufs=2)
                        for k in range(GRP):
                            c = c0 + k
                            ks = slice(k * RPC, (k + 1) * RPC)
                            nc.scalar.activation(
                                a_t[:, ks], sbc_all[:, cs], AF.Exp,
                                bias=whb[:, c * GW + GW - 1:c * GW + GW])
                            nc.vector.tensor_scalar_mul(
                                b_t[:, ks], e02bc_all[:, cs], ed02[:, c:c + 1])
                        m_t = ew.tile([128, GRP * RPC], CDT, tag="m", bufs=2)
                        nc.vector.tensor_max(m_t[:], a_t[:], b_t[:])
                        p_t = ew.tile([128, GRP * RPC], CDT, tag="p", bufs=3)
                        meng.tensor_mul(
                            p_t[:], m_t[:], adj_sb[:, c0 * RPC:(c0 + GRP) * RPC])
                        for k in range(GRP):
                            c = c0 + k
                            for t in range(NJT):
                                _mm(nc, pa[t][:],
                                    p_t[:, k * RPC + t * 128:k * RPC + (t + 1) * 128],
                                    whb[:, c * GW:c * GW + SW],
                                    start=(c == 0), stop=(c == NCH - 1))
                    # normalize + ELU straight into hfull; reciprocal reads
                    # the denominator column straight from PSUM (one DVE op)
                    for t in range(NJT):
                        rden = small.tile([128, 1], F32, tag="rden")
                        nc.vector.reciprocal(rden[:], pa[t][:, 0:1])
                        y = hfull[t][:, h * HID:(h + 1) * HID]
                        nc.scalar.mul(y, pa[t][:, 1:SW], rden[:])
                        q = ew.tile([128, HID], CDT, tag="q")
                        nc.scalar.activation(q[:], y, AF.Exp)
                        nc.vector.tensor_scalar(q[:], q[:], -1.0, 0.0,
                                                mybir.AluOpType.add,
                                                mybir.AluOpType.min)
                        nc.vector.tensor_max(y, y, q[:])

                # software-pipelined: Wh of head pivot+h overlaps att of
                # head h, so the elementwise engines pace the whole layer
                # while PE stays busy
                def transpose_chunk(kc):
                    lo, sz = kch_o[kc]
                    psT = ps.tile([128, RPC], F32, tag=f"p{6 + kc % 2}",
                                  name=f"psT{kc}")
                    for t in range(NJT):
                        _mm(nc, psT[:sz, t * 128:(t + 1) * 128],
                            hfull[t][:, lo:lo + sz], eye_sb[:],
                            start=True, stop=True)
                    nc.scalar.activation(ht[kc][:sz, :], psT[:sz, :], AF.Copy)

                for h in range(min(pivot, HEADS)):
                    wh_head(h)
                for h in range(HEADS):
                    att_head(h)
                    if pivot + h < HEADS:
                        wh_head(pivot + h)

                # --- transpose h [i, o] -> h^T chunk tiles ---
                for kc in range(len(kch_o)):
                    transpose_chunk(kc)

            # ---------------- output attention layer + pool + MLP ----------
            def _tail():
                nkc = len(kch_o)
                wlo = dram.tile([RPC, GWO], CDT, name="wlo")
                wgo = dram.tile([N, GWO], CDT, name="wgo", addr_space="Shared")
                nsp = [(0, 512), (512, WO2 - 512)]
                stride = WO2 + 1       # ones|Wh(600)|d|s
                wlo_sb = small.tile([128, NJT * stride], CDT, tag="wlo_sb")
                # ci-outer: each Ro chunk is streamed ONCE and consumed by
                # all 3 jt accumulators (6 PSUM banks p0-p5 held), instead
                # of re-streaming the full 3.6MB Ro three times
                pswo = [[ps.tile([128, sz], F32, tag=f"p{jt * 2 + si}",
                                 name=f"pswo{jt}_{si}")
                         for si, (lo, sz) in enumerate(nsp)]
                        for jt in range(NJT)]
                for ci, (lo, sz) in enumerate(kch_o):
                    r_t = rstream.tile([128, WO2], CDT, tag="ro", bufs=6)
                    nc.sync.dma_start(r_t[:sz, :], Ro[lo:lo + sz, :])
                    for jt in range(NJT):
                        for si, (slo, ssz) in enumerate(nsp):
                            _mm(nc, pswo[jt][si][:],
                                ht[ci][:sz, jt * 128:(jt + 1) * 128],
                                r_t[:sz, slo:slo + ssz],
                                start=(ci == 0), stop=(ci == nkc - 1))
                for jt in range(NJT):
                    base = jt * stride
                    nc.vector.memset(wlo_sb[:, base:base + 1], 1.0)
                    for si, (slo, ssz) in enumerate(nsp):
                        nc.scalar.activation(
                            wlo_sb[:, base + 1 + slo:base + 1 + slo + ssz],
                            pswo[jt][si][:], AF.Copy)
                    nc.sync.dma_start(wlo[jt * 128:(jt + 1) * 128, :],
                                      wlo_sb[:, base:base + GWO])
                s_transpose_and_bcast(wlo_sb, stride, 0)
                nc.gpsimd.collective_compute(
                    "AllGather", mybir.AluOpType.bypass, replica_groups=rg,
                    ins=[wlo.opt()], outs=[wgo.opt()])

                cs = slice(0, RPC)
                onspl = [(0, 512), (512, SWO - 512)]   # slices of ones|Wh
                ps_o = [[ps.tile([128, sz], F32, tag=f"p{t * 2 + si}",
                                 name=f"pso{t}_{si}")
                         for si, (lo, sz) in enumerate(onspl)]
                        for t in range(NJT)]
                for c0 in range(0, NCH, GRP):
                    g = c0 // GRP
                    meng = nc.vector if g % 3 == 2 else nc.gpsimd
                    whcs = []
                    a_t = ew.tile([128, GRP * RPC], CDT, tag="a", bufs=2)
                    b_t = ew.tile([128, GRP * RPC], CDT, tag="b", bufs=2)
                    for k in range(GRP):
                        c = c0 + k
                        ks = slice(k * RPC, (k + 1) * RPC)
                        whc = rstream.tile([128, GWO], CDT, tag=f"whc{k}",
                                           name=f"whc{c}", bufs=2)
                        nc.scalar.dma_start(whc[:],
                                            wgo[c * 128:(c + 1) * 128, :])
                        whcs.append(whc)
                        ed02c = small.tile([128, 1], F32, tag="ed02c", bufs=3)
                        nc.scalar.activation(ed02c[:], whc[:, GWO - 1:GWO],
                                             AF.Exp, scale=ALPHA)
                        nc.scalar.activation(
                            a_t[:, ks], sbc_all[:, cs], AF.Exp,
                            bias=whc[:, GWO - 1:GWO])
                        nc.vector.tensor_scalar_mul(
                            b_t[:, ks], e02bc_all[:, cs], ed02c[:])
                    m_t = ew.tile([128, GRP * RPC], CDT, tag="m", bufs=2)
                    nc.vector.tensor_max(m_t[:], a_t[:], b_t[:])
                    p_t = ew.tile([128, GRP * RPC], CDT, tag="p", bufs=3)
                    meng.tensor_mul(p_t[:], m_t[:],
                                    adj_sb[:, c0 * RPC:(c0 + GRP) * RPC])
                    for k in range(GRP):
                        c = c0 + k
                        for t in range(NJT):
                            for si, (slo, ssz) in enumerate(onspl):
                                _mm(nc, ps_o[t][si][:],
                                    p_t[:, k * RPC + t * 128:k * RPC + (t + 1) * 128],
                                    whcs[k][:, slo:slo + ssz],
                                    start=(c == 0), stop=(c == NCH - 1))

                # normalize + ELU; att_tiles[t]: [128, 600]
                att_tiles = []
                for t in range(NJT):
                    rdeno = small.tile([128, 1], F32, tag="rdeno")
                    nc.vector.reciprocal(rdeno[:], ps_o[t][0][:, 0:1])
                    att_sb = ew.tile([128, OUT_ATT], CDT, tag=f"atts{t}",
                                     name=f"atts{t}", bufs=1)
                    nc.vector.tensor_scalar_mul(att_sb[:, 0:511],
                                                ps_o[t][0][:, 1:512], rdeno[:])
                    nc.vector.tensor_scalar_mul(att_sb[:, 511:OUT_ATT],
                                                ps_o[t][1][:, 0:SWO - 512],
                                                rdeno[:])
                    q = ew.tile([128, OUT_ATT], CDT, tag="qo")
                    nc.scalar.activation(q[:], att_sb[:], AF.Exp)
                    nc.vector.tensor_scalar(q[:], q[:], -1.0, 0.0,
                                            mybir.AluOpType.add,
                                            mybir.AluOpType.min)
                    nc.vector.tensor_max(att_sb[:], att_sb[:], q[:])
                    if DEBUG_STAGE == "HO":
                        a32 = ew.tile([128, OUT_ATT], F32, tag="a32dbg")
                        nc.vector.tensor_copy(a32[:], att_sb[:])
                        nc.sync.dma_start(dbg[t * 128:(t + 1) * 128, :], a32[:])
                    att_tiles.append(att_sb)

                # per-core slot pools [NSLOT, 600] -> AllGather -> combine
                pool_l = dram.tile([NSLOT, OUT_ATT], F32, name="pool_l")
                pool_g = dram.tile([NCORES * NSLOT, OUT_ATT], F32,
                                   name="pool_g", addr_space="Shared")
                psl = [(0, 512), (512, OUT_ATT - 512)]
                pq_sb = small.tile([NSLOT, OUT_ATT], F32, tag="pq_sb")
                for si, (slo, ssz) in enumerate(psl):
                    psq = ps.tile([NSLOT, ssz], F32, tag=f"p{6 + si}",
                                  name=f"psq{si}")
                    for t in range(NJT):
                        _mm(nc, psq[:], smat_sb[t][:],
                            att_tiles[t][:, slo:slo + ssz],
                            start=(t == 0), stop=(t == NJT - 1))
                    nc.scalar.activation(pq_sb[:, slo:slo + ssz], psq[:],
                                         AF.Copy)
                nc.sync.dma_start(pool_l[:], pq_sb[:])
                nc.gpsimd.collective_compute(
                    "AllGather", mybir.AluOpType.bypass, replica_groups=rg,
                    ins=[pool_l.opt()], outs=[pool_g.opt()])

                # ---------------- MLP (replicated) ----------------
                pg16 = persist.tile([128, OUT_ATT], F32, name="pg16")
                nc.sync.dma_start(pg16[:], pool_g[:])
                pg_sb = []
                for g, (glo, gsz) in enumerate(gsp):
                    psm = ps.tile([128, N_GRAPHS], F32, tag=f"p{g % 2}",
                                  name=f"psg{g}")
                    nc.tensor.matmul(psm[:gsz, :], pg16[:, glo:glo + gsz],
                                     cmat_sb[:], start=True, stop=True)
                    t = persist.tile([128, N_GRAPHS], CDT, name=f"pg{g}")
                    nc.vector.tensor_copy(t[:gsz, :], psm[:gsz, :])
                    pg_sb.append(t)
                if DEBUG_STAGE == "POOL":
                    for g, (glo, gsz) in enumerate(gsp):
                        t32 = small.tile([128, N_GRAPHS], F32, tag="pooldbg")
                        nc.vector.tensor_copy(t32[:gsz, :], pg_sb[g][:gsz, :])
                        nc.sync.dma_start(dbg[glo:glo + gsz, :], t32[:gsz, :])
                h1_sb = []
                for m, (mlo, msz) in enumerate(msp):
                    psm = ps.tile([128, N_GRAPHS], F32, tag=f"p{m % 2}")
                    for g, (glo, gsz) in enumerate(gsp):
                        _mm(nc, psm[:msz, :], wm1_sb[g][:gsz, mlo:mlo + msz],
                            pg_sb[g][:gsz, :], start=(g == 0),
                            stop=(g == len(gsp) - 1))
                    t = persist.tile([128, N_GRAPHS], CDT, name=f"h1_{m}")
                    nc.scalar.activation(t[:msz, :], psm[:msz, :], AF.Relu,
                                         bias=bm1_sb[m][:msz, :])
                    h1_sb.append(t)
                for o, (olo, osz) in enumerate(_chunks(NOUT)):
                    b2 = small.tile([128, 1], F32, tag="bm2")
                    nc.sync.dma_start(b2[:osz, :], bm2[olo:olo + osz, :])
                    psm = ps.tile([128, N_GRAPHS], F32, tag=f"p{2 + o % 2}")
                    for m, (mlo, msz) in enumerate(msp):
                        _mm(nc, psm[:osz, :], wm2_sb[m][:msz, olo:olo + osz],
                            h1_sb[m][:msz, :], start=(m == 0),
                            stop=(m == len(msp) - 1))
                    ot = small.tile([128, N_GRAPHS], F32, tag="ot")
                    nc.vector.tensor_scalar_add(ot[:osz, :], psm[:osz, :],
                                                b2[:osz, :])
                    nc.sync.dma_start(outT[olo:olo + osz, :], ot[:osz, :])

            # ---------------- run the stages ----------------
            def dump_ht():
                for kc, (lo, sz) in enumerate(kch_o):
                    t32 = rstream.tile([128, RPC], F32, tag="tdb32",
                                       name=f"t32{kc}")
                    nc.vector.tensor_copy(t32[:sz, :], ht[kc][:sz, :])
                    nc.sync.dma_start(dbg[lo:lo + sz, :], t32[:sz, :])

            gat_layer(0, F_IN, R0, [1, 1, 2, 3, 3], 10)
            stop = False
            if DEBUG_STAGE == "L0":
                dump_ht()
                stop = True
            if not stop:
                gat_layer(1, FO, R1, [1, 1, 2, 3, 3], 5)
                if DEBUG_STAGE == "L1":
                    dump_ht()
                    stop = True
            if not stop:
                _tail()

    nc.compile()
    return nc


# ======================= host side =======================

def _np_cdt(a):
    import ml_dtypes
    return np.ascontiguousarray(np.asarray(a, np.float32).astype(ml_dtypes.bfloat16))


def _prep_inputs(x, edge_index, batch, W0, a0_src, a0_dst, W1, a1_src, a1_dst,
                 W_out, ao_src, ao_dst, Wm1, bm1, Wm2, bm2):
    x = np.asarray(x, np.float32)
    ei = np.asarray(edge_index)
    batch = np.asarray(batch).astype(np.int64)
    adj = np.zeros((N, N), np.float32)
    adj[ei[0], ei[1]] = 1.0

    cnt = np.bincount(batch, minlength=N_GRAPHS).astype(np.float32)
    cnt = np.maximum(cnt, 1.0)
    # per-core slot pooling: core c's rows span graphs [lo_c, lo_c+NSLOT)
    lo_cs, smat16s = [], []
    cmat_np = np.zeros((NCORES * NSLOT, N_GRAPHS), np.float32)
    for c in range(NCORES):
        b = batch[c * RPC:(c + 1) * RPC]
        lo = int(b.min())
        assert int(b.max()) - lo + 1 <= NSLOT, "graph span exceeds NSLOT"
        lo_cs.append(lo)
        sm = np.zeros((RPC, NSLOT), np.float32)
        sm[np.arange(RPC), b - lo] = 1.0 / cnt[b]
        smat16s.append(sm)
        for k in range(NSLOT):
            g = lo + k
            if g < N_GRAPHS:
                cmat_np[c * NSLOT + k, g] = 1.0

    W0 = np.asarray(W0, np.float32)
    W1 = np.asarray(W1, np.float32)
    W_out = np.asarray(W_out, np.float32)

    def fuse(W, a_dst, a_src):   # [H,F,O],[H,O],[H,O] -> [H,F,O+2]
        wad = np.einsum('hfo,ho->hf', W, np.asarray(a_dst, np.float32))
        was = np.einsum('hfo,ho->hf', W, np.asarray(a_src, np.float32))
        return np.concatenate([W, wad[:, :, None], was[:, :, None]], axis=2)

    R0p = fuse(W0, a0_dst, a0_src)
    R1p = fuse(W1, a1_dst, a1_src)
    Rop = np.concatenate(
        [W_out, (W_out @ np.asarray(ao_dst, np.float32))[:, None],
         (W_out @ np.asarray(ao_src, np.float32))[:, None]], axis=1)

    shared = dict(
        R0=_np_cdt(R0p), R1=_np_cdt(R1p), Ro=_np_cdt(Rop),
        Wm1=_np_cdt(Wm1),
        bm1=np.ascontiguousarray(np.asarray(bm1, np.float32)[:, None]),
        Wm2=_np_cdt(Wm2),
        bm2=np.ascontiguousarray(np.asarray(bm2, np.float32)[:, None]),
        eye128=_np_cdt(np.eye(128, dtype=np.float32)),
        cmat=np.ascontiguousarray(cmat_np),
    )
    xT_full = x.T
    in_maps = []
    for c in range(NCORES):
        rows = slice(c * RPC, (c + 1) * RPC)
        m = dict(shared)
        m["xT"] = _np_cdt(xT_full[:, rows])
        m["adjT"] = _np_cdt(adj[rows, :].T)
        m["smat16"] = _np_cdt(smat16s[c])
        in_maps.append(m)
    return in_maps


_last_results = None


def kernel(**inputs):
    global _last_results
    if "k" not in _compiled:
        _compiled["k"] = build()
    nc = _compiled["k"]
    in_maps = _prep_inputs(**inputs)
    kw = {}
    if TRACE:
        try:
            import tracehook
            tracehook.install()
            kw = dict(trace=True)
            td = os.environ.get("KERNEL_TRACEDIR")
            if td:
                kw["tmpdir"] = td
        except ImportError:
            pass
    res = run_bass_kernel_spmd(nc, in_maps, core_ids=list(range(NCORES)), **kw)
    _last_results = res
    return np.ascontiguousarray(res.results[0]["outT"].T)

